# revision 36
# baseline (speedup 1.0000x reference)
"""BKT (Bayesian Knowledge Tracing) forward pass on Trainium2, 8 NeuronCores.

The reference's chunked 32-trajectory scan is a 2-state HMM forward pass.
Per (sequence, t):  W_t = diag(o_t) @ Tr  (2x2 per-step matrix),
    alpha' = alpha @ W_t,   p_corr(t) = (alpha . pc_t) / (alpha . 1),
    out = [Ln(1-p), Ln(p)].
Alphas are kept UNNORMALIZED pairs throughout (scale cancels in the
prediction ratio); only occasional power-limited renorms keep fp range.

Device algorithm per core (2048 seqs = 128 partitions x 16 groups, layout
[p, t, s, s', g] with g innermost so every bf16 op hits the DVE 2x mode):
  1. Pool: zpk = lls2 * cm (sign-fold; host pre-negates the slip logit so a
     single sigmoid yields both obs probs).  ACT: op = sigmoid(zpk) -> bf16,
     pc = sigmoid(lls2) -> fp32.
  2. DVE bf16: W = op x Tr; chunk products A_c over K=10 steps (2 fused
     broadcast tensor_tensor per fold, parallel across chunks).
  3. Pool fp32: serial alpha-pair recursion over chunk matrices (2 ops per
     chunk); DVE renorm (approx-recip) every 2 chunks bounds the range.
  4. DVE bf16: within-chunk alpha recovery (1 mult + 1 add per step).
  5. fp32 predictions: qp = alpha*pc, num/den sums, approx-recip, q = 1-p
     (fp32 keeps the cancellation harmless), Ln on ACT, bf16 store (host
     upcasts, which halves the output traffic).

Sharding: pure data-parallel over batch; the tiny per-KC/per-problem tables
are gathered on host (traffic-neutral marshaling), recurrences on device.
"""

import numpy as np

import concourse.bass as bass
import concourse.bacc as bacc
import concourse.tile as tile
import concourse.mybir as mybir

F32 = mybir.dt.float32
BF16 = mybir.dt.bfloat16
I8 = mybir.dt.int8
AF = mybir.ActivationFunctionType
OP = mybir.AluOpType

P = 128
N_CORES = 8


def emit_bkt(nc, G, T, K, SEG, RN=4):
    assert T % SEG == 0 and SEG % K == 0
    NSEG = T // SEG
    CS = SEG // K
    CT = T // K

    lls_d = nc.dram_tensor("lls2", [P, T, 2, G], F32, kind="ExternalInput")
    zlq_d = nc.dram_tensor("zlq", [P, T, 2, G], F32, kind="ExternalInput")
    dyn_d = nc.dram_tensor("dyn", [P, 3, G], F32, kind="ExternalInput")
    out_d = nc.dram_tensor("out", [P, T, 2, G], BF16, kind="ExternalOutput")

    with tile.TileContext(nc) as tc:
        with (
            tc.tile_pool(name="singles", bufs=1) as singles,
            tc.tile_pool(name="io", bufs=3) as io,
            tc.tile_pool(name="wk2", bufs=3) as wk2,
            tc.tile_pool(name="wk1", bufs=1) as wk1,
        ):
            # ---- per-sequence constants ----
            dyn_t = singles.tile([P, 3, G], F32)
            nc.sync.dma_start(dyn_t[:], dyn_d[:])
            # Tr[s][s'][g]: [[1-l, l], [f, 1-f]]
            Tp = singles.tile([P, 2, 2, G], BF16)
            nc.scalar.activation(Tp[:, 0, 0], dyn_t[:, 0, :], AF.Sigmoid, scale=-1.0)
            nc.scalar.activation(Tp[:, 0, 1], dyn_t[:, 0, :], AF.Sigmoid)
            nc.scalar.activation(Tp[:, 1, 0], dyn_t[:, 1, :], AF.Sigmoid)
            nc.scalar.activation(Tp[:, 1, 1], dyn_t[:, 1, :], AF.Sigmoid, scale=-1.0)
            # chunk-start alpha pairs (unnormalized); start = (1-pI0, pI0)
            starts = singles.tile([P, CT + 1, 2, G], F32)
            nc.scalar.activation(starts[:, 0, 0], dyn_t[:, 2, :], AF.Sigmoid, scale=-1.0)
            nc.scalar.activation(starts[:, 0, 1], dyn_t[:, 2, :], AF.Sigmoid)

            obs = {}
            fin = {}

            def phase_a(seg, nsplit=1):
                s0 = seg * SEG
                lls = io.tile([P, SEG, 2, G], F32, tag="lls")
                zlq = io.tile([P, SEG, 2, G], F32, tag="zlq")
                op_t = wk2.tile([P, SEG, 2, G], BF16, tag="op")
                pc_t = wk2.tile([P, SEG, 2, G], F32, tag="pc")
                bounds = [SEG * h // nsplit for h in range(nsplit + 1)]
                # zlq first: op sigmoids gate the fold pipeline; pc is only
                # needed much later (predictions)
                for h in range(nsplit):
                    a, b = bounds[h], bounds[h + 1]
                    nc.sync.dma_start(zlq[:, a:b], zlq_d[:, s0 + a : s0 + b])
                    nc.scalar.activation(op_t[:, a:b], zlq[:, a:b], AF.Sigmoid)
                for h in range(nsplit):
                    a, b = bounds[h], bounds[h + 1]
                    nc.sync.dma_start(lls[:, a:b], lls_d[:, s0 + a : s0 + b])
                    nc.scalar.activation(pc_t[:, a:b], lls[:, a:b], AF.Sigmoid)
                obs[seg] = (op_t, pc_t)

            def finalize(seg):
                s0 = seg * SEG
                out_t = fin.pop(seg)
                h = SEG // 2
                nc.sync.dma_start(out_d[:, s0 : s0 + h], out_t[:, :h])
                nc.sync.dma_start(out_d[:, s0 + h : s0 + SEG], out_t[:, h:])

            def phase_b(seg):
                c0 = seg * CS
                op_t, pc_t = obs.pop(seg)

                # W[t][s][s'][g] = op_s(t) * Tr[s][s']  (g innermost: 2x mode)
                Wp = wk2.tile([P, SEG, 2, 2, G], BF16, tag="Wp")
                nc.vector.tensor_tensor(
                    Wp[:],
                    op_t[:].unsqueeze(3).broadcast_to((P, SEG, 2, 2, G)),
                    Tp[:].unsqueeze(1).broadcast_to((P, SEG, 2, 2, G)),
                    OP.mult,
                )
                Wc = Wp[:].rearrange("p (c k) s t g -> p c k s t g", k=K)

                if seg >= 1:
                    finalize(seg - 1)

                # chunk products A_c = W_c0 @ ... @ W_c,K-1 ([c, i, s', g]);
                # every prefix product is also saved to Pref ([i, c, jj, s',
                # g]) which de-chains the within-chunk recovery below.
                A = wk2.tile([P, CS, 2, 2, G], BF16, tag="A", bufs=4)
                TM = wk2.tile([P, CS, 2, 2, 2, G], BF16, tag="TM", bufs=2)
                Pref = wk2.tile([P, 2, CS, K - 1, 2, G], BF16, tag="Pref")

                def save_pref(jj, src):
                    nc.vector.tensor_scalar(
                        Pref[:, :, :, jj],
                        src.rearrange("p c i t g -> p i c t g"),
                        1.0, 0.0, OP.mult, OP.add,
                    )

                nc.vector.tensor_scalar(A[:], Wc[:, :, 0], 1.0, 0.0, OP.mult, OP.add)
                save_pref(0, A[:])
                for j in range(1, K):
                    nc.vector.tensor_tensor(
                        TM[:],
                        A[:].unsqueeze(4).broadcast_to((P, CS, 2, 2, 2, G)),
                        Wc[:, :, j].unsqueeze(2).broadcast_to((P, CS, 2, 2, 2, G)),
                        OP.mult,
                    )
                    nc.vector.tensor_tensor(A[:], TM[:, :, :, 0], TM[:, :, :, 1], OP.add)
                    if j < K - 1:
                        save_pref(j, A[:])

                # serial alpha-pair recursion over chunks (Pool, fp32).
                # High priority keeps the latency-critical chain ahead of
                # bulk Pool work in the scheduler.
                sv = wk1.tile([P, 2, 2, G], F32, tag="sv")
                rcp = wk1.tile([P, G], F32, tag="rcp")
                for cl in range(CS):
                    cg = c0 + cl
                    with tc.high_priority():
                        nc.gpsimd.tensor_tensor(
                            sv[:],
                            starts[:, cg].unsqueeze(2).broadcast_to((P, 2, 2, G)),
                            A[:, cl], OP.mult,
                        )
                        nc.gpsimd.tensor_tensor(
                            starts[:, cg + 1], sv[:, 0], sv[:, 1], OP.add
                        )
                    if cg % RN == RN - 1:
                        # natural (late) priority: the in-order DVE queue
                        # must not park on this tiny dependent op
                        nc.vector.reciprocal_approx_fast(
                            rcp[:], starts[:, cg + 1, 0]
                        )
                        nc.vector.tensor_tensor(
                            starts[:, cg + 1],
                            starts[:, cg + 1],
                            rcp[:].unsqueeze(1).broadcast_to((P, 2, G)),
                            OP.mult,
                        )

                # within-chunk alpha recovery, de-chained via prefixes:
                # a_j = a_0 @ P_j for all j at once (3 big bf16 ops)
                rec = wk2.tile([P, CS, K, 2, G], BF16, tag="rec")
                with tc.high_priority():
                    nc.gpsimd.tensor_scalar(
                        rec[:, :, 0], starts[:, c0 : c0 + CS], 1.0, 0.0, OP.mult, OP.add
                    )
                PM0 = wk1.tile([P, CS, K - 1, 2, G], BF16, tag="PM0")
                PM1 = wk1.tile([P, CS, K - 1, 2, G], BF16, tag="PM1")
                JT = (K - 1) * 2
                for i, PM in ((0, PM0), (1, PM1)):
                    nc.vector.tensor_tensor(
                        PM[:].rearrange("p c k t g -> p c (k t) g"),
                        rec[:, :, 0, i].unsqueeze(2).broadcast_to((P, CS, JT, G)),
                        Pref[:, i].rearrange("p c k t g -> p c (k t) g"),
                        OP.mult,
                    )
                nc.vector.tensor_tensor(
                    rec[:, :, 1:], PM0[:], PM1[:], OP.add,
                )

                # predictions (fp32): p = (a.pc)/(a.1), q = 1-p.
                # Halved for pipelining; num->qp0, pt->qp1, qt->den reuse
                # keeps the fp32 scratch to four tiles.
                af = rec[:].rearrange("p c k s g -> p (c k) s g")
                qp0 = wk1.tile([P, SEG, G], F32, tag="qp0")
                qp1 = wk1.tile([P, SEG, G], F32, tag="qp1")
                den = wk1.tile([P, SEG, G], F32, tag="den")
                rdn = wk1.tile([P, SEG // 2, G], F32, tag="rdn")
                out_t = io.tile([P, SEG, 2, G], BF16, tag="out")
                # last segment: nothing left to hide Pool latency behind, so
                # keep its predictions on DVE and use finer slices
                last = seg == NSEG - 1
                peng = nc.vector if last else nc.gpsimd
                nq = 4 if last else 2
                bnds = [SEG * hh // nq for hh in range(nq + 1)]
                for a, b in zip(bnds[:-1], bnds[1:]):
                    peng.tensor_tensor(
                        qp0[:, a:b], af[:, a:b, 0], pc_t[:, a:b, 0], OP.mult)
                    peng.tensor_tensor(
                        qp1[:, a:b], af[:, a:b, 1], pc_t[:, a:b, 1], OP.mult)
                    peng.tensor_tensor(
                        den[:, a:b], af[:, a:b, 0], af[:, a:b, 1], OP.add)
                    nc.vector.tensor_tensor(
                        qp0[:, a:b], qp0[:, a:b], qp1[:, a:b], OP.add)
                    nc.vector.reciprocal_approx_fast(rdn[:, : b - a], den[:, a:b])
                    nc.vector.tensor_tensor(
                        qp1[:, a:b], qp0[:, a:b], rdn[:, : b - a], OP.mult)
                    nc.scalar.activation(
                        den[:, a:b], qp1[:, a:b], AF.Identity, bias=1.0, scale=-1.0)
                    nc.scalar.activation(out_t[:, a:b, 0], den[:, a:b], AF.Ln)
                    nc.scalar.activation(out_t[:, a:b, 1], qp1[:, a:b], AF.Ln)
                fin[seg] = out_t

            for seg in range(NSEG):
                phase_a(seg, nsplit=(2 if seg == 0 else 1))
                if seg >= 1:
                    phase_b(seg - 1)
            phase_b(NSEG - 1)
            finalize(NSEG - 1)

    return nc


# ------------------------------------------------------------------
# Host-side full-problem wrapper
# ------------------------------------------------------------------

_B, _T, _K, _SEG = 16384, 500, 10, 50
_G = _B // (P * N_CORES)

_cached = {}


def _build():
    if "nc" not in _cached:
        nc = bacc.Bacc(None, target_bir_lowering=False)
        emit_bkt(nc, G=_G, T=_T, K=_K, SEG=_SEG)
        nc.compile()
        _cached["nc"] = nc
    return _cached["nc"]


def _shard(arr, core):
    """(B,...) -> this core's (P, ..., G) permuted view, seq = g*128 + p."""
    rows = arr[core * P * _G : (core + 1) * P * _G]
    r = rows.reshape(_G, P, *arr.shape[1:])
    order = (1,) + tuple(range(2, r.ndim)) + (0,)
    return np.ascontiguousarray(r.transpose(order))


def kernel(corr, kc, problem, dynamics_logits_table, obs_logits_kc,
           obs_logits_problem, fastbkt_n):
    from concourse.bass_utils import run_bass_kernel_spmd

    corr = np.asarray(corr, dtype=np.float32)
    kc = np.asarray(kc).astype(np.int64)
    problem = np.asarray(problem).astype(np.int64)
    dyn_table = np.asarray(dynamics_logits_table, dtype=np.float32)
    obs_kc = np.asarray(obs_logits_kc, dtype=np.float32)
    obs_prob = np.asarray(obs_logits_problem, dtype=np.float32)

    B, T = corr.shape
    assert B == _B and T == _T, (B, T)

    # host gathers (input marshaling); slip logit pre-negated, and the
    # corr sign-fold is applied host-side (paid for in DMA bytes)
    lls = obs_kc[kc][:, None, :] + obs_prob[problem]       # (B, T, 2)
    lls[:, :, 1] *= -1.0                                   # [lg, -ls]
    zlq = lls * (corr * 2.0 - 1.0)[:, :, None]             # sign-folded
    dyn = dyn_table[kc]                                    # (B, 3)

    nc = _build()
    in_maps = []
    for core in range(N_CORES):
        in_maps.append({
            "lls2": _shard(lls, core),
            "zlq": _shard(zlq.astype(np.float32), core),
            "dyn": _shard(dyn, core),
        })

    res = run_bass_kernel_spmd(
        nc, in_maps, core_ids=list(range(N_CORES)), **_cached.get("run_kwargs", {})
    )
    _cached["last_results"] = res

    out = np.empty((B, T, 2), np.float32)
    for core in range(N_CORES):
        o = np.asarray(res.results[core]["out"]).astype(np.float32)  # (P,T,2,G)
        rows = o.transpose(3, 0, 1, 2).reshape(P * _G, T, 2)
        out[core * P * _G : (core + 1) * P * _G] = rows
    return out


# revision 39
# speedup vs baseline: 1.0598x; 1.0598x over previous
"""BKT (Bayesian Knowledge Tracing) forward pass on Trainium2, 8 NeuronCores.

The reference's chunked 32-trajectory scan is a 2-state HMM forward pass.
Per (sequence, t):  W_t = diag(o_t) @ Tr  (2x2 per-step matrix),
    alpha' = alpha @ W_t,   p_corr(t) = (alpha . pc_t) / (alpha . 1),
    out = [Ln(1-p), Ln(p)].
Alphas are kept UNNORMALIZED pairs throughout (scale cancels in the
prediction ratio); only occasional power-limited renorms keep fp range.

Device algorithm per core (2048 seqs = 128 partitions x 16 groups, layout
[p, t, s, s', g] with g innermost so every bf16 op hits the DVE 2x mode):
  1. Pool: zpk = lls2 * cm (sign-fold; host pre-negates the slip logit so a
     single sigmoid yields both obs probs).  ACT: op = sigmoid(zpk) -> bf16,
     pc = sigmoid(lls2) -> fp32.
  2. DVE bf16: W = op x Tr; chunk products A_c over K=10 steps (2 fused
     broadcast tensor_tensor per fold, parallel across chunks).
  3. Pool fp32: serial alpha-pair recursion over chunk matrices (2 ops per
     chunk); DVE renorm (approx-recip) every 2 chunks bounds the range.
  4. DVE bf16: within-chunk alpha recovery (1 mult + 1 add per step).
  5. fp32 predictions: qp = alpha*pc, num/den sums, approx-recip, q = 1-p
     (fp32 keeps the cancellation harmless), Ln on ACT, bf16 store (host
     upcasts, which halves the output traffic).

Sharding: pure data-parallel over batch; the tiny per-KC/per-problem tables
are gathered on host (traffic-neutral marshaling), recurrences on device.
"""

import numpy as np

import concourse.bass as bass
import concourse.bacc as bacc
import concourse.tile as tile
import concourse.mybir as mybir

F32 = mybir.dt.float32
BF16 = mybir.dt.bfloat16
I8 = mybir.dt.int8
AF = mybir.ActivationFunctionType
OP = mybir.AluOpType

P = 128
N_CORES = 8


def emit_bkt(nc, G, T, K, SEG, RN=4):
    assert T % SEG == 0 and SEG % K == 0
    NSEG = T // SEG
    CS = SEG // K
    CT = T // K

    lls_d = nc.dram_tensor("lls2", [P, T, 2, G], F32, kind="ExternalInput")
    zlq_d = nc.dram_tensor("zlq", [P, T, 2, G], F32, kind="ExternalInput")
    dyn_d = nc.dram_tensor("dyn", [P, 3, G], F32, kind="ExternalInput")
    out_d = nc.dram_tensor("out", [P, T, 2, G], BF16, kind="ExternalOutput")

    with tile.TileContext(nc) as tc:
        with (
            tc.tile_pool(name="singles", bufs=1) as singles,
            tc.tile_pool(name="io", bufs=2) as io,
            tc.tile_pool(name="wk2", bufs=2) as wk2,
            tc.tile_pool(name="wk1", bufs=1) as wk1,
        ):
            # ---- per-sequence constants ----
            dyn_t = singles.tile([P, 3, G], F32)
            nc.sync.dma_start(dyn_t[:], dyn_d[:])
            # Tr[s][s'][g]: [[1-l, l], [f, 1-f]]
            Tp = singles.tile([P, 2, 2, G], BF16)
            nc.scalar.activation(Tp[:, 0, 0], dyn_t[:, 0, :], AF.Sigmoid, scale=-1.0)
            nc.scalar.activation(Tp[:, 0, 1], dyn_t[:, 0, :], AF.Sigmoid)
            nc.scalar.activation(Tp[:, 1, 0], dyn_t[:, 1, :], AF.Sigmoid)
            nc.scalar.activation(Tp[:, 1, 1], dyn_t[:, 1, :], AF.Sigmoid, scale=-1.0)
            # chunk-start alpha pairs (unnormalized); start = (1-pI0, pI0)
            starts = singles.tile([P, CT + 1, 2, G], F32)
            nc.scalar.activation(starts[:, 0, 0], dyn_t[:, 2, :], AF.Sigmoid, scale=-1.0)
            nc.scalar.activation(starts[:, 0, 1], dyn_t[:, 2, :], AF.Sigmoid)

            obs = {}
            fin = {}

            def phase_a(seg, nsplit=1):
                s0 = seg * SEG
                lls = io.tile([P, SEG, 2, G], F32, tag="lls")
                zlq = io.tile([P, SEG, 2, G], F32, tag="zlq")
                op_t = wk2.tile([P, SEG, 2, G], BF16, tag="op")
                pc_t = wk2.tile([P, SEG, 2, G], F32, tag="pc")
                bounds = [SEG * h // nsplit for h in range(nsplit + 1)]
                # zlq first: op sigmoids gate the fold pipeline; pc is only
                # needed much later (predictions)
                for h in range(nsplit):
                    a, b = bounds[h], bounds[h + 1]
                    nc.sync.dma_start(zlq[:, a:b], zlq_d[:, s0 + a : s0 + b])
                    nc.scalar.activation(op_t[:, a:b], zlq[:, a:b], AF.Sigmoid)
                for h in range(nsplit):
                    a, b = bounds[h], bounds[h + 1]
                    nc.sync.dma_start(lls[:, a:b], lls_d[:, s0 + a : s0 + b])
                    nc.scalar.activation(pc_t[:, a:b], lls[:, a:b], AF.Sigmoid)
                obs[seg] = (op_t, pc_t)

            def finalize(seg):
                s0 = seg * SEG
                out_t = fin.pop(seg)
                h = SEG // 2
                nc.sync.dma_start(out_d[:, s0 : s0 + h], out_t[:, :h])
                nc.sync.dma_start(out_d[:, s0 + h : s0 + SEG], out_t[:, h:])

            def phase_b(seg):
                c0 = seg * CS
                op_t, pc_t = obs.pop(seg)

                # W[t][s][s'][g] = op_s(t) * Tr[s][s']  (g innermost: 2x mode)
                Wp = wk2.tile([P, SEG, 2, 2, G], BF16, tag="Wp")
                nc.vector.tensor_tensor(
                    Wp[:],
                    op_t[:].unsqueeze(3).broadcast_to((P, SEG, 2, 2, G)),
                    Tp[:].unsqueeze(1).broadcast_to((P, SEG, 2, 2, G)),
                    OP.mult,
                )
                Wc = Wp[:].rearrange("p (c k) s t g -> p c k s t g", k=K)

                if seg >= 1:
                    finalize(seg - 1)

                # chunk products A_c = W_c0 @ ... @ W_c,K-1 ([c, i, s', g]);
                # every prefix product is also saved to Pref ([i, c, jj, s',
                # g]) which de-chains the within-chunk recovery below.
                A = wk2.tile([P, CS, 2, 2, G], BF16, tag="A", bufs=3)
                TM = wk2.tile([P, CS, 2, 2, 2, G], BF16, tag="TM", bufs=2)
                Pref = wk2.tile([P, 2, CS, K - 1, 2, G], BF16, tag="Pref", bufs=3)

                def save_pref(jj, src):
                    nc.vector.tensor_scalar(
                        Pref[:, :, :, jj],
                        src.rearrange("p c i t g -> p i c t g"),
                        1.0, 0.0, OP.mult, OP.add,
                    )

                nc.vector.tensor_scalar(A[:], Wc[:, :, 0], 1.0, 0.0, OP.mult, OP.add)
                save_pref(0, A[:])
                for j in range(1, K):
                    nc.vector.tensor_tensor(
                        TM[:],
                        A[:].unsqueeze(4).broadcast_to((P, CS, 2, 2, 2, G)),
                        Wc[:, :, j].unsqueeze(2).broadcast_to((P, CS, 2, 2, 2, G)),
                        OP.mult,
                    )
                    nc.vector.tensor_tensor(A[:], TM[:, :, :, 0], TM[:, :, :, 1], OP.add)
                    if j < K - 1:
                        save_pref(j, A[:])

                # serial alpha-pair recursion over chunks (Pool, fp32).
                # High priority keeps the latency-critical chain ahead of
                # bulk Pool work in the scheduler.
                sv = wk1.tile([P, 2, 2, G], F32, tag="sv")
                rcp = wk1.tile([P, G], F32, tag="rcp")
                for cl in range(CS):
                    cg = c0 + cl
                    with tc.high_priority():
                        nc.gpsimd.tensor_tensor(
                            sv[:],
                            starts[:, cg].unsqueeze(2).broadcast_to((P, 2, 2, G)),
                            A[:, cl], OP.mult,
                        )
                        nc.gpsimd.tensor_tensor(
                            starts[:, cg + 1], sv[:, 0], sv[:, 1], OP.add
                        )
                    if cg % RN == RN - 1:
                        # natural (late) priority: the in-order DVE queue
                        # must not park on this tiny dependent op
                        nc.vector.reciprocal_approx_fast(
                            rcp[:], starts[:, cg + 1, 0]
                        )
                        nc.vector.tensor_tensor(
                            starts[:, cg + 1],
                            starts[:, cg + 1],
                            rcp[:].unsqueeze(1).broadcast_to((P, 2, G)),
                            OP.mult,
                        )

                # within-chunk alpha recovery, de-chained via prefixes:
                # a_j = a_0 @ P_j for all j at once (3 big bf16 ops)
                rec = wk2.tile([P, CS, K, 2, G], BF16, tag="rec")
                with tc.high_priority():
                    nc.gpsimd.tensor_scalar(
                        rec[:, :, 0], starts[:, c0 : c0 + CS], 1.0, 0.0, OP.mult, OP.add
                    )
                PM1 = wk1.tile([P, CS, K - 1, 2, G], BF16, tag="PM1")
                JT = (K - 1) * 2
                recv = rec[:, :, 1:].rearrange("p c k t g -> p c (k t) g")
                nc.vector.tensor_tensor(
                    recv,
                    rec[:, :, 0, 0].unsqueeze(2).broadcast_to((P, CS, JT, G)),
                    Pref[:, 0].rearrange("p c k t g -> p c (k t) g"),
                    OP.mult,
                )
                nc.vector.tensor_tensor(
                    PM1[:].rearrange("p c k t g -> p c (k t) g"),
                    rec[:, :, 0, 1].unsqueeze(2).broadcast_to((P, CS, JT, G)),
                    Pref[:, 1].rearrange("p c k t g -> p c (k t) g"),
                    OP.mult,
                )
                nc.vector.tensor_tensor(
                    rec[:, :, 1:], rec[:, :, 1:], PM1[:], OP.add,
                )

                # predictions (fp32): p = (a.pc)/(a.1), q = 1-p.
                # Halved for pipelining; num->qp0, pt->qp1, qt->den reuse
                # keeps the fp32 scratch to four tiles.
                af = rec[:].rearrange("p c k s g -> p (c k) s g")
                out_t = io.tile([P, SEG, 2, G], BF16, tag="out")
                # last segment: nothing left to hide Pool latency behind, so
                # keep its predictions on DVE and use finer slices
                last = seg == NSEG - 1
                peng = nc.vector if last else nc.gpsimd
                nq = 4
                bnds = [SEG * hh // nq for hh in range(nq + 1)]
                for a, b in zip(bnds[:-1], bnds[1:]):
                    w = b - a
                    qp0 = wk1.tile([P, w, G], F32, tag="qp0", bufs=2)
                    qp1 = wk1.tile([P, w, G], F32, tag="qp1", bufs=2)
                    den = wk1.tile([P, w, G], F32, tag="den", bufs=2)
                    rdn = wk1.tile([P, w, G], F32, tag="rdn", bufs=2)
                    peng.tensor_tensor(
                        qp0[:], af[:, a:b, 0], pc_t[:, a:b, 0], OP.mult)
                    peng.tensor_tensor(
                        qp1[:], af[:, a:b, 1], pc_t[:, a:b, 1], OP.mult)
                    peng.tensor_tensor(
                        den[:], af[:, a:b, 0], af[:, a:b, 1], OP.add)
                    nc.vector.tensor_tensor(qp0[:], qp0[:], qp1[:], OP.add)
                    nc.vector.reciprocal_approx_fast(rdn[:], den[:])
                    nc.vector.tensor_tensor(qp1[:], qp0[:], rdn[:], OP.mult)
                    nc.scalar.activation(
                        den[:], qp1[:], AF.Identity, bias=1.0, scale=-1.0)
                    nc.scalar.activation(out_t[:, a:b, 0], den[:], AF.Ln)
                    nc.scalar.activation(out_t[:, a:b, 1], qp1[:], AF.Ln)
                fin[seg] = out_t

            for seg in range(NSEG):
                phase_a(seg, nsplit=(4 if seg == 0 else 1))
                if seg >= 1:
                    phase_b(seg - 1)
            phase_b(NSEG - 1)
            finalize(NSEG - 1)

    return nc


# ------------------------------------------------------------------
# Host-side full-problem wrapper
# ------------------------------------------------------------------

_B, _T, _K, _SEG = 16384, 500, 10, 100
_G = _B // (P * N_CORES)

_cached = {}


def _build():
    if "nc" not in _cached:
        nc = bacc.Bacc(None, target_bir_lowering=False)
        emit_bkt(nc, G=_G, T=_T, K=_K, SEG=_SEG)
        nc.compile()
        _cached["nc"] = nc
    return _cached["nc"]


def _shard(arr, core):
    """(B,...) -> this core's (P, ..., G) permuted view, seq = g*128 + p."""
    rows = arr[core * P * _G : (core + 1) * P * _G]
    r = rows.reshape(_G, P, *arr.shape[1:])
    order = (1,) + tuple(range(2, r.ndim)) + (0,)
    return np.ascontiguousarray(r.transpose(order))


def kernel(corr, kc, problem, dynamics_logits_table, obs_logits_kc,
           obs_logits_problem, fastbkt_n):
    from concourse.bass_utils import run_bass_kernel_spmd

    corr = np.asarray(corr, dtype=np.float32)
    kc = np.asarray(kc).astype(np.int64)
    problem = np.asarray(problem).astype(np.int64)
    dyn_table = np.asarray(dynamics_logits_table, dtype=np.float32)
    obs_kc = np.asarray(obs_logits_kc, dtype=np.float32)
    obs_prob = np.asarray(obs_logits_problem, dtype=np.float32)

    B, T = corr.shape
    assert B == _B and T == _T, (B, T)

    # host gathers (input marshaling); slip logit pre-negated, and the
    # corr sign-fold is applied host-side (paid for in DMA bytes)
    lls = obs_kc[kc][:, None, :] + obs_prob[problem]       # (B, T, 2)
    lls[:, :, 1] *= -1.0                                   # [lg, -ls]
    zlq = lls * (corr * 2.0 - 1.0)[:, :, None]             # sign-folded
    dyn = dyn_table[kc]                                    # (B, 3)

    nc = _build()
    in_maps = []
    for core in range(N_CORES):
        in_maps.append({
            "lls2": _shard(lls, core),
            "zlq": _shard(zlq.astype(np.float32), core),
            "dyn": _shard(dyn, core),
        })

    res = run_bass_kernel_spmd(
        nc, in_maps, core_ids=list(range(N_CORES)), **_cached.get("run_kwargs", {})
    )
    _cached["last_results"] = res

    out = np.empty((B, T, 2), np.float32)
    for core in range(N_CORES):
        o = np.asarray(res.results[core]["out"]).astype(np.float32)  # (P,T,2,G)
        rows = o.transpose(3, 0, 1, 2).reshape(P * _G, T, 2)
        out[core * P * _G : (core + 1) * P * _G] = rows
    return out
